# revision 6
# baseline (speedup 1.0000x reference)
"""Multi-head attention (B=2, Q=K=2048, H=16, D=V=64) on 8 Trainium2 cores.

Sharding: batch x heads. Core c handles batch b = c//4 and heads
[4*(c%4), 4*(c%4)+4) -- 4 (b,h) "pairs" per core, no cross-core comm.

Device algorithm per (b,h) pair (flash-style, no max subtraction needed:
scores are ~N(0,1) so exp() is far from fp32 overflow; the reference's
max-subtraction cancels exactly in the softmax ratio up to a vanishing
eps*exp(-max) term ~1e-12 relative):

  for each q-block (1024 wide):
    for each k-chunk (128 keys):
      S^T[k,q] = (K-chunk d,k)^T @ (Q^T d,q)   on TensorE (bf16 in, fp32 acc)
      E = exp(S/8)                              on ScalarE, PSUM -> SBUF bf16
      acc[0:65, q] += V''^T @ E                 on TensorE (V'' = [V*mask | mask])
    row 64 of acc = sum_k mask*E (denominator), rows 0..63 = unnormalized O^T
    O^T = acc[0:64] * broadcast(1/(acc[64]+eps))  (DVE recip + GPSIMD bcast)

K-chunks alternate PE array row-halves (even chunks rows 0-63, odd rows
64-127) so each chunk's LDWEIGHTS targets a different row group than the
in-flight matmuls and pulls ahead; Q^T is duplicated into both partition
halves to feed either row group.

Host does layout only: transposes Q/K to [d, seq], interleaves K chunks,
reshapes V/mask, transposes O^T back on unshard.
"""

import os
import sys

import numpy as np

sys.path.insert(0, "/opt/trn_rl_repo")

import concourse.bacc as bacc
import concourse.mybir as mybir
import concourse.tile as tile
from concourse.bass_utils import run_bass_kernel_spmd

N_CORES = 8
B, Q, K, H, D, V = 2, 2048, 2048, 16, 64, 64
PAIRS = 4            # (b,h) pairs per core
KC = K // 128        # 16 k-chunks of 128 keys
QBW = 1024           # q-block width
QB = Q // QBW        # 2 q-blocks
EPS = 1e-10

F32 = mybir.dt.float32
BF16 = mybir.dt.bfloat16
I32 = mybir.dt.int32

_cached_nc = None
LAST_RESULTS = None


def _build_program():
    nc = bacc.Bacc("TRN2", target_bir_lowering=False, debug=False, num_devices=N_CORES)

    # qT: per pair [64(d), Q];  kTe: per pair even/odd-interleaved chunks
    # [128(d x 2), KC//2, 128];  v: per pair k-chunked;  maskT: [128, KC]
    qT = nc.dram_tensor("qT", [PAIRS, 64, Q], F32, kind="ExternalInput").ap()
    kTe = nc.dram_tensor("kTe", [PAIRS, 128, KC // 2, 128], F32, kind="ExternalInput").ap()
    v = nc.dram_tensor("v", [PAIRS, KC, 128, V], F32, kind="ExternalInput").ap()
    maskT = nc.dram_tensor("maskT", [128, KC], I32, kind="ExternalInput").ap()
    o = nc.dram_tensor("o", [PAIRS, V, Q], F32, kind="ExternalOutput").ap()

    with tile.TileContext(nc) as tc:
        with (
            tc.sbuf_pool(name="persist", bufs=1) as persist,
            tc.sbuf_pool(name="staging", bufs=2) as staging,
            tc.sbuf_pool(name="epool", bufs=3) as epool,
            tc.sbuf_pool(name="norm", bufs=2) as normp,
            tc.psum_pool(name="win", bufs=2) as winp,
            tc.psum_pool(name="acc", bufs=2) as accp,
        ):
            # ---------------- input prep ----------------
            mask_i = staging.tile([128, KC], I32, tag="mask_i")
            nc.sync.dma_start(out=mask_i, in_=maskT)
            mask_f = persist.tile([128, KC], F32, tag="mask_f")
            nc.vector.tensor_copy(out=mask_f, in_=mask_i)
            mask_b = persist.tile([128, KC], BF16, tag="mask_b")
            nc.vector.tensor_copy(out=mask_b, in_=mask_f)

            qTb, kTb, vpp = [], [], []
            for p in range(PAIRS):
                st = staging.tile([128, Q], F32, tag="q_stage")
                nc.sync.dma_start(out=st[0:64, :], in_=qT[p])
                nc.sync.dma_start(out=st[64:128, :], in_=qT[p])
                qb = persist.tile([128, Q], BF16, tag=f"qTb{p}")
                nc.vector.tensor_copy(out=qb, in_=st)
                qTb.append(qb)

                st = staging.tile([128, KC // 2, 128], F32, tag="k_stage")
                nc.sync.dma_start(out=st, in_=kTe[p])
                kb = persist.tile([128, KC // 2, 128], BF16, tag=f"kTb{p}")
                nc.vector.tensor_copy(out=kb, in_=st)
                kTb.append(kb)

                # V'' : [128, KC, 65] bf16, cols 0..63 = V*mask, col 64 = mask
                vt = persist.tile([128, KC, V + 1], BF16, tag=f"vpp{p}")
                nc.vector.tensor_copy(out=vt[:, :, V], in_=mask_b)
                for c in range(KC):
                    vs = staging.tile([128, V], F32, tag="v_stage")
                    nc.sync.dma_start(out=vs, in_=v[p, c])
                    nc.vector.tensor_scalar(
                        out=vt[:, c, 0:V],
                        in0=vs,
                        scalar1=mask_f[:, c : c + 1],
                        scalar2=None,
                        op0=mybir.AluOpType.mult,
                    )
                vpp.append(vt)

            # ---------------- main loops ----------------
            for p in range(PAIRS):
                for blk in range(QB):
                    qcols = slice(blk * QBW, (blk + 1) * QBW)
                    acc = accp.tile([V + 1, QBW], F32, tag="acc")
                    for c in range(KC):
                        half = c % 2
                        hrows = slice(64 * half, 64 * half + 64)
                        lhsT = kTb[p][hrows, c // 2, :]
                        win = winp.tile([128, QBW], F32, tag="win")
                        for j in range(QBW // 512):
                            nc.tensor.matmul(
                                win[:, j * 512 : (j + 1) * 512],
                                lhsT,
                                qTb[p][hrows, blk * QBW + j * 512 : blk * QBW + (j + 1) * 512],
                                start=True,
                                stop=True,
                            )
                        e = epool.tile([128, QBW], BF16, tag="e")
                        nc.scalar.activation(
                            out=e,
                            in_=win,
                            func=mybir.ActivationFunctionType.Exp,
                            scale=0.125,
                        )
                        for j in range(QBW // 512):
                            nc.tensor.matmul(
                                acc[:, j * 512 : (j + 1) * 512],
                                vpp[p][:, c, :],
                                e[:, j * 512 : (j + 1) * 512],
                                start=(c == 0),
                                stop=(c == KC - 1),
                            )
                    # normalize: rows 0..63 divided by (row 64 + eps)
                    deps = normp.tile([1, QBW], F32, tag="deps")
                    nc.vector.tensor_scalar_add(
                        out=deps, in0=acc[V : V + 1, :], scalar1=EPS
                    )
                    rec = normp.tile([1, QBW], F32, tag="rec")
                    nc.vector.reciprocal(out=rec, in_=deps)
                    recb = normp.tile([64, QBW], F32, tag="recb")
                    nc.gpsimd.partition_broadcast(recb, rec)
                    oT = normp.tile([64, QBW], F32, tag="oT")
                    nc.vector.tensor_mul(out=oT, in0=acc[0:V, :], in1=recb)
                    nc.sync.dma_start(out=o[p, :, qcols], in_=oT)

    nc.compile()
    return nc


def _get_program():
    global _cached_nc
    if _cached_nc is None:
        _cached_nc = _build_program()
    return _cached_nc


def _shard_inputs(queries, keys, values, key_mask):
    queries = np.asarray(queries, dtype=np.float32)
    keys = np.asarray(keys, dtype=np.float32)
    values = np.asarray(values, dtype=np.float32)
    key_mask = np.asarray(key_mask, dtype=np.int32)

    # [B, S, H, D] -> [B, H, D, S]
    qT_full = np.ascontiguousarray(queries.transpose(0, 2, 3, 1))
    kT_full = np.ascontiguousarray(keys.transpose(0, 2, 3, 1))

    in_maps = []
    for core in range(N_CORES):
        b, h0 = core // 4, (core % 4) * 4
        # even/odd interleave of kT chunks: [64, K] -> [64, KC, 128]
        # -> rows 0-63 = even chunks, rows 64-127 = odd chunks
        kc = kT_full[b, h0 : h0 + 4].reshape(PAIRS, 64, KC, 128)
        kTe = np.concatenate([kc[:, :, 0::2, :], kc[:, :, 1::2, :]], axis=1)
        in_maps.append(
            {
                "qT": np.ascontiguousarray(qT_full[b, h0 : h0 + 4]),
                "kTe": np.ascontiguousarray(kTe),
                "v": np.ascontiguousarray(
                    values[b, :, h0 : h0 + 4, :]
                    .transpose(1, 0, 2)
                    .reshape(PAIRS, KC, 128, V)
                ),
                "maskT": np.ascontiguousarray(key_mask[b].reshape(KC, 128).T),
            }
        )
    return in_maps


def kernel(queries, keys, values, key_mask):
    global LAST_RESULTS
    nc = _get_program()
    in_maps = _shard_inputs(queries, keys, values, key_mask)
    res = run_bass_kernel_spmd(nc, in_maps, list(range(N_CORES)))
    LAST_RESULTS = res

    out = np.empty((B, Q, H * V), dtype=np.float32)
    for core in range(N_CORES):
        b, h0 = core // 4, (core % 4) * 4
        oc = res.results[core]["o"]  # [PAIRS, V, Q]
        for p in range(PAIRS):
            h = h0 + p
            out[b, :, h * V : (h + 1) * V] = oc[p].T
    return out
